# revision 1
# baseline (speedup 1.0000x reference)
"""BitLinear (BitNet b1.58 ternary-weight linear) Trainium2 kernel, 8-core SPMD.

Reference computation:
    gamma = max(mean(|W|), 1e-8)
    QW    = clip(round(W / gamma), -1, 1)          # in {-1, 0, 1}
    out   = x @ QW.T + bias                        # x: [4, 2048, 4096] f32

Sharding (2 x 4 grid over 8 cores):
    - x   split in half along the (flattened) batch axis M=8192 -> M_loc=4096,
      transposed on host to xT [K, M_loc] so the contraction dim lands on
      SBUF partitions.
    - W   split in 4 along out_features N=4096 -> N_loc=1024, transposed on
      host to wT [K, N_loc].  Each W shard is held by 2 cores (the two m-halves).
    - gamma needs mean(|W|) over the FULL W: each core abs-sums its local wT
      shard, a 1-element AllReduce sums across cores (each W element is counted
      exactly twice -> fold the 1/2 into the threshold constant).
    - Quantization uses  clip(round(w/g), -1, 1) == (sign(w - g/2) + sign(w + g/2)) / 2
      (exact except on the measure-zero set w == +-g/2), evaluated as two ScalarE
      Sign activations + one VectorE add producing q2 = 2*qw in bf16; the 1/2 is
      absorbed into x's f32->bf16 cast (x * 0.5).
    - out[m, n] = sum_k (0.5*x[m,k]) * (2*qw[n,k]) + bias[n], accumulated in
      f32 PSUM over 32 k-tiles, bias added from a host-broadcast [128, N_loc]
      tile on the way out.

kernel(**inputs) takes the full unsharded inputs and returns the full output.
Host work is layout only (transpose / slice / broadcast / concat); all
arithmetic runs on the NeuronCores.
"""

import numpy as np

N_CORES = 8
GRID_M, GRID_N = 2, 4          # core c -> (mi, ni) = (c // GRID_N, c % GRID_N)

B, S, K, N = 4, 2048, 4096, 4096
M = B * S                      # 8192
M_LOC = M // GRID_M            # 4096
N_LOC = N // GRID_N            # 1024
TJ = K // 128                  # 32 k-tiles
TCH = 4                        # k-tiles per w-stream chunk
MM_N = 512                     # matmul moving free dim (one PSUM bank of f32)

# threshold const: AR sums |W| with every element counted GRID_M times.
# th = gamma/2 = max(mean/2, 5e-9),  mean = AR / (GRID_M * K * N)
TH_SCALE = 1.0 / (2.0 * GRID_M * K * N)
TH_FLOOR = 0.5e-8


def split_multi_waits(nc, limit=1):
    """The walrus build in this container supports only `limit` sync-waits on
    CTRL-type (Drain/NoOp) instructions, but Tile's exit barrier attaches one
    wait per outstanding processor.  Split the extras onto preceding
    single-wait NOPs on the same engine (waits execute in issue order on the
    sequencer, so this is semantically identical)."""
    import concourse.mybir as mybir

    n_split = 0
    for f in nc.m.functions:
        for b in f.blocks:
            out_list = []
            changed = False
            for ins in b.instructions:
                si = getattr(ins, "sync_info", None)
                ow = list(si.on_wait) if (si is not None and si.on_wait) else []
                if len(ow) > limit:
                    for j, w in enumerate(ow[:-limit]):
                        nop = mybir.InstNoOp(name=f"{ins.name}-ws{j}")
                        nop.engine = ins.engine
                        nop.sync_info = mybir.SyncInfo(on_wait=[w], on_update=[])
                        out_list.append(nop)
                        n_split += 1
                    si.on_wait = ow[-limit:]
                    changed = True
                out_list.append(ins)
            if changed:
                b.instructions = out_list
    return n_split


def dedup_ldweights(nc):
    """Tile lowers every matmul into an explicit Ldweights + Matmult pair, so
    two consecutive matmuls sharing one stationary tile reload the PE array
    twice.  Drop an Ldweights when the instruction directly before it is a
    Matmult whose stationary operand is byte-identical and the Ldweights
    carries no semaphore waits/updates — the weights are already in the
    array."""
    n_drop = 0
    for f in nc.m.functions:
        for b in f.blocks:
            insts = list(b.instructions)
            out_list = []
            for ins in insts:
                if (type(ins).__name__ == "InstLdweights"
                        and out_list
                        and type(out_list[-1]).__name__ == "InstMatmult"
                        and len(out_list[-1].ins) >= 2
                        and str(out_list[-1].ins[1]) == str(ins.ins[0])
                        and not (ins.sync_info and ins.sync_info.on_wait)
                        and not (ins.sync_info and ins.sync_info.on_update)):
                    n_drop += 1
                    continue
                out_list.append(ins)
            if n_drop:
                b.instructions = out_list
    return n_drop


def build_nc(m_loc=M_LOC, k=K, n_loc=N_LOC, tch=TCH, n_cores=N_CORES,
             grid_m=GRID_M, split_waits=True, repeat_b=1, stage="full",
             mm_n=MM_N, dedup_ldw=True, repeat_a=1):
    """Build the per-core Bass graph (SPMD: identical on every core)."""
    import concourse.bass as bass
    import concourse.mybir as mybir
    import concourse.tile as tile

    f32 = mybir.dt.float32
    bf16 = mybir.dt.bfloat16
    Alu = mybir.AluOpType
    Act = mybir.ActivationFunctionType

    tj = k // 128
    tch = min(tch, tj)
    assert tj % tch == 0
    m_tiles = m_loc // 128
    n_half = (n_loc + mm_n - 1) // mm_n
    ng = n_loc // grid_m            # gamma-slice width (disjoint across cores)
    # AR over the disjoint wg slices sums |W| exactly once; th = gamma/2
    th_scale = 1.0 / (2.0 * k * (n_loc * (n_cores // grid_m)))

    nc = bass.Bass(num_devices=n_cores)
    # xt is host-pre-tiled: xt[mi, p, t*128+j] = x_loc[mi*128+j, t*128+p]
    # so each m-tile's load is one fully-contiguous [128, tj*128] block
    # (16 KiB runs per partition instead of 512 B strided rows).
    xt = nc.dram_tensor("xt", [m_tiles, 128, tj * 128], f32,
                        kind="ExternalInput")
    wt = nc.dram_tensor("wt", [k, n_loc], f32, kind="ExternalInput")
    wg = nc.dram_tensor("wg", [k, ng], f32, kind="ExternalInput")
    biasb = nc.dram_tensor("biasb", [128, n_loc], f32, kind="ExternalInput")
    out = nc.dram_tensor("out", [m_loc, n_loc], f32, kind="ExternalOutput")

    cc_in = nc.dram_tensor("cc_in", [1], f32, kind="Internal")
    cc_out = nc.dram_tensor("cc_out", [1], f32, kind="Internal",
                            addr_space="Shared")

    wt_r = wt[:, :].rearrange("(t p) n -> p t n", p=128)
    wg_r = wg[:, :].rearrange("(t p) n -> p t n", p=128)

    with tile.TileContext(nc) as tc:
        with (
            tc.tile_pool(name="const", bufs=1) as constp,
            tc.tile_pool(name="gam", bufs=1) as gamp,
            tc.tile_pool(name="wch", bufs=2) as wchp,
            tc.tile_pool(name="qtmp", bufs=2) as qtmpp,
            tc.tile_pool(name="q2", bufs=1) as q2p,
            tc.tile_pool(name="xin", bufs=3) as xinp,
            tc.tile_pool(name="xbf", bufs=2) as xbfp,
            tc.tile_pool(name="osb", bufs=3) as osbp,
            tc.tile_pool(name="ps", bufs=3, space="PSUM") as psp,
            tc.tile_pool(name="ps_small", bufs=1, space="PSUM") as pssp,
        ):
            # ---- constants ----
            biasb_sb = constp.tile([128, n_loc], f32, tag="biasb")
            nc.sync.dma_start(biasb_sb[:], biasb[:, :])
            ones_col = constp.tile([128, 1], f32, tag="ones_col")
            nc.vector.memset(ones_col[:], 1.0)
            ones_row = constp.tile([1, 128], f32, tag="ones_row")
            nc.vector.memset(ones_row[:], 1.0)

            # ---- phase A: gamma = max(mean|W|, 1e-8) ----
            for _ra in range(repeat_a):
                acc = gamp.tile([128, tj], f32, tag="acc")
                for ci in range(tj // tch):
                    gch = wchp.tile([128, tch * ng], f32, tag="gch")
                    gch3 = gch[:].rearrange("p (t n) -> p t n", n=ng)
                    for tt in range(tch):
                        nc.sync.dma_start(gch3[:, tt, :],
                                          wg_r[:, ci * tch + tt, :])
                    for tt in range(tch):
                        t = ci * tch + tt
                        nc.vector.tensor_reduce(
                            acc[:, t:t + 1], gch3[:, tt, :],
                            axis=mybir.AxisListType.X, op=Alu.add,
                            apply_absolute_value=True)
                acc1 = gamp.tile([128, 1], f32, tag="acc1")
                nc.vector.tensor_reduce(acc1[:], acc[:],
                                        axis=mybir.AxisListType.X, op=Alu.add)
                # cross-partition sum -> [1, 1]
                ps1 = pssp.tile([1, 1], f32, tag="ps1")
                nc.tensor.matmul(ps1[:], lhsT=acc1[:], rhs=ones_col[:],
                                 start=True, stop=True)
                s_sb = gamp.tile([1, 1], f32, tag="s_sb")
                nc.vector.tensor_copy(s_sb[:], ps1[:])
                nc.sync.dma_start(cc_in[0:1], s_sb[0:1, 0])
                cc = nc.gpsimd.collective_compute(
                    "AllReduce", Alu.add,
                    replica_groups=[list(range(n_cores))],
                    ins=[cc_in.ap().opt()], outs=[cc_out.ap().opt()])
                s2_sb = gamp.tile([1, 1], f32, tag="s2_sb")
                rd = nc.sync.dma_start(s2_sb[0:1, 0], cc_out[0:1])
                tile.add_dep_helper(rd.ins, cc.ins, reason="read AR result")
                # broadcast to all 128 partitions
                psb = pssp.tile([128, 1], f32, tag="psb")
                nc.tensor.matmul(psb[:], lhsT=ones_row[:], rhs=s2_sb[:],
                                 start=True, stop=True)
                th = gamp.tile([128, 1], f32, tag="th")
                nth = gamp.tile([128, 1], f32, tag="nth")
                nc.vector.tensor_scalar(th[:], psb[:], th_scale, TH_FLOOR,
                                        op0=Alu.mult, op1=Alu.max)
                nc.vector.tensor_scalar(nth[:], psb[:], -th_scale, -TH_FLOOR,
                                        op0=Alu.mult, op1=Alu.min)

                # ---- phase A2: quantize W -> q2 = 2*qw (bf16, resident) ----
                q2 = q2p.tile([128, tj * n_loc], bf16, tag="q2")
                q2_3 = q2[:].rearrange("p (t n) -> p t n", n=n_loc)
                for ci in range(tj // tch):
                    wch = wchp.tile([128, tch * n_loc], f32, tag="wg")
                    wch3 = wch[:].rearrange("p (t n) -> p t n", n=n_loc)
                    for tt in range(tch):
                        nc.sync.dma_start(wch3[:, tt, :],
                                          wt_r[:, ci * tch + tt, :])
                    for tt in range(tch):
                        t = ci * tch + tt
                        a = qtmpp.tile([128, n_loc], bf16, tag="qa")
                        b = qtmpp.tile([128, n_loc], bf16, tag="qb")
                        nc.scalar.activation(a[:], wch3[:, tt, :], Act.Sign,
                                             bias=nth[:], scale=1.0)
                        nc.scalar.activation(b[:], wch3[:, tt, :], Act.Sign,
                                             bias=th[:], scale=1.0)
                        nc.vector.tensor_tensor(q2_3[:, t, :], a[:], b[:],
                                                op=Alu.add)

            # ---- phase B: out = (0.5 x)T q2 + bias, streamed over m-tiles ----
            for _rep in range(repeat_b if stage != "prologue" else 0):
                for mi in range(m_tiles):
                    xraw = xinp.tile([128, tj * 128], f32, tag="xraw")
                    xraw3 = xraw[:].rearrange("p (t j) -> p t j", j=128)
                    # split the 2 MB tile load over 8 DMA queues
                    xq = min(8, tj)
                    step = (tj * 128) // xq
                    for c in range(xq):
                        nc.sync.dma_start(
                            xraw[:, c * step:(c + 1) * step],
                            xt[mi, :, c * step:(c + 1) * step])
                    xbf = xbfp.tile([128, tj * 128], bf16, tag="xbf")
                    xbf3 = xbf[:].rearrange("p (t j) -> p t j", j=128)
                    nc.scalar.activation(xbf[:], xraw[:],
                                         Act.Copy, scale=0.5)
                    osb = osbp.tile([128, n_loc], f32, tag="osb")
                    if stage == "full":
                        ps = psp.tile([128, n_loc], f32, tag="ps")
                        for t in range(tj):
                            for h in range(n_half):
                                n0 = h * mm_n
                                n1 = min(n_loc, n0 + mm_n)
                                nc.tensor.matmul(ps[:, n0:n1],
                                                 lhsT=xbf3[:, t, :],
                                                 rhs=q2_3[:, t, n0:n1],
                                                 start=(t == 0),
                                                 stop=(t == tj - 1))
                        nc.vector.tensor_tensor(osb[:], ps[:], biasb_sb[:],
                                                op=Alu.add)
                    else:
                        nc.vector.tensor_tensor(osb[:], xbf[:, 0:n_loc],
                                                biasb_sb[:], op=Alu.add)
                    nc.sync.dma_start(out[mi * 128:(mi + 1) * 128, :], osb[:])

    if dedup_ldw:
        dedup_ldweights(nc)
    if split_waits:
        split_multi_waits(nc)
    return nc


def shard_inputs(x, weight, bias, m_loc=M_LOC, n_loc=N_LOC, n_cores=N_CORES,
                 grid_n=GRID_N):
    """Host-side layout prep (transpose/slice/broadcast only)."""
    x2 = np.ascontiguousarray(x.reshape(-1, x.shape[-1]))     # [M, K]
    k = x2.shape[1]
    m_tiles, tj = m_loc // 128, k // 128
    grid_m = n_cores // grid_n
    ng = n_loc // grid_m
    in_maps = []
    xts = {}
    for c in range(n_cores):
        mi, ni = c // grid_n, c % grid_n
        if mi not in xts:
            # xt[mi, p, t*128+j] = x_loc[mi*128+j, t*128+p]
            xl = x2[mi * m_loc:(mi + 1) * m_loc, :]
            xts[mi] = np.ascontiguousarray(
                xl.reshape(m_tiles, 128, tj, 128)
                .transpose(0, 3, 2, 1)
                .reshape(m_tiles, 128, tj * 128))
        wt = np.ascontiguousarray(weight[ni * n_loc:(ni + 1) * n_loc, :].T)
        g0 = ni * n_loc + mi * ng
        wgt = np.ascontiguousarray(weight[g0:g0 + ng, :].T)
        bb = np.ascontiguousarray(
            np.broadcast_to(bias[ni * n_loc:(ni + 1) * n_loc], (128, n_loc)))
        in_maps.append({"xt": xts[mi], "wt": wt, "wg": wgt, "biasb": bb})
    return in_maps


def unshard_output(outs, x_shape, m_loc=M_LOC, n_loc=N_LOC, n_cores=N_CORES,
                   grid_m=GRID_M, grid_n=GRID_N):
    n = grid_n * n_loc
    full = np.empty((grid_m * m_loc, n), dtype=outs[0].dtype)
    for c in range(n_cores):
        mi, ni = c // grid_n, c % grid_n
        full[mi * m_loc:(mi + 1) * m_loc, ni * n_loc:(ni + 1) * n_loc] = outs[c]
    return full.reshape(*x_shape[:-1], n)


def kernel(x, weight, bias):
    from concourse.bass_utils import run_bass_kernel_spmd

    nc = build_nc()
    in_maps = shard_inputs(x, weight, bias)
    res = run_bass_kernel_spmd(nc, in_maps, core_ids=list(range(N_CORES)))
    outs = [res.results[c]["out"] for c in range(N_CORES)]
    return unshard_output(outs, x.shape)



# revision 15
# speedup vs baseline: 42.0405x; 42.0405x over previous
"""BitLinear (BitNet b1.58 ternary-weight linear) Trainium2 kernel, 8-core SPMD.

Reference computation:
    gamma = max(mean(|W|), 1e-8)
    QW    = clip(round(W / gamma), -1, 1)          # in {-1, 0, 1}
    out   = x @ QW.T + bias                        # x: [4, 2048, 4096] f32

Sharding (2 x 4 grid over 8 cores):
    - x   split in half along the (flattened) batch axis M=8192 -> M_loc=4096,
      transposed on host to xT [K, M_loc] so the contraction dim lands on
      SBUF partitions.
    - W   split in 4 along out_features N=4096 -> N_loc=1024, transposed on
      host to wT [K, N_loc].  Each W shard is held by 2 cores (the two m-halves).
    - gamma needs mean(|W|) over the FULL W: each core abs-sums a disjoint
      [K, N_loc/2] slice (wg), a 1-element AllReduce sums across cores.
    - Quantization uses  clip(round(w/g), -1, 1) == (sign(w - g/2) + sign(w + g/2)) / 2
      (exact except on the measure-zero set w == +-g/2), evaluated as two ScalarE
      Sign activations + one VectorE add producing q2 = 2*qw in bf16; the 1/2 is
      absorbed into x's f32->bf16 cast (x * 0.5).
    - out[m, n] = sum_k (0.5*x[m,k]) * (2*qw[n,k]) + bias[n], accumulated in
      f32 PSUM over 32 k-tiles, bias added from a host-broadcast [128, N_loc]
      tile on the way out.

Schedule (what makes it fast; cost model gives ~520 us/core, PE busy 436 us
= the bf16 matmul roofline for this shape):
    - gamma phase: chunk-wide wg DMAs on the SP HWDGE stream (the modeled
      ~330 GB/s HBM pipe) with 3 bufs so a pool-slot WAR never opens a gap
      for wt/x prefetch DMAs to jump the line; |.|-sums alternate between
      DVE tensor_reduce and ACT Abs+accum_out so reduction never paces DMA.
    - the 1-element AllReduce handoff DMAs + bias/output stores ride the
      second (ACT) HWDGE stream; wt chunks + first x tiles prefetch under
      the collective's ~28 us latency.
    - phase B's first k-sweep interleaves `interleave` m-tiles' PSUM
      accumulation chains so the PE consumes each just-quantized q2 k-tile
      several times while A2 is still producing (Sign pairs at ~3.6 us per
      2 k-tiles); remaining m-tiles run k-inner back-to-back, PE-bound.

build_nc(repeat_full=R) repeats the whole kernel body R times inside one
graph; test.py differences two repeat counts to measure per-kernel HW time
without the multi-ms axon/PJRT dispatch overhead.

kernel(**inputs) takes the full unsharded inputs and returns the full output.
Host work is layout only (transpose / slice / broadcast / concat); all
arithmetic runs on the NeuronCores.
"""

import numpy as np

N_CORES = 8
GRID_M, GRID_N = 2, 4          # core c -> (mi, ni) = (c // GRID_N, c % GRID_N)

B, S, K, N = 4, 2048, 4096, 4096
M = B * S                      # 8192
M_LOC = M // GRID_M            # 4096
N_LOC = N // GRID_N            # 1024
TJ = K // 128                  # 32 k-tiles
TCH = 4                        # k-tiles per w-stream chunk
MM_N = 512                     # matmul moving free dim (one PSUM bank of f32)

# threshold const: AR sums |W| with every element counted GRID_M times.
# th = gamma/2 = max(mean/2, 5e-9),  mean = AR / (GRID_M * K * N)
TH_SCALE = 1.0 / (2.0 * GRID_M * K * N)
TH_FLOOR = 0.5e-8


def split_multi_waits(nc, limit=1):
    """The walrus build in this container supports only `limit` sync-waits on
    CTRL-type (Drain/NoOp) instructions, but Tile's exit barrier attaches one
    wait per outstanding processor.  Split the extras onto preceding
    single-wait NOPs on the same engine (waits execute in issue order on the
    sequencer, so this is semantically identical)."""
    import concourse.mybir as mybir

    n_split = 0
    for f in nc.m.functions:
        for b in f.blocks:
            out_list = []
            changed = False
            for ins in b.instructions:
                si = getattr(ins, "sync_info", None)
                ow = list(si.on_wait) if (si is not None and si.on_wait) else []
                if len(ow) > limit:
                    for j, w in enumerate(ow[:-limit]):
                        nop = mybir.InstNoOp(name=f"{ins.name}-ws{j}")
                        nop.engine = ins.engine
                        nop.sync_info = mybir.SyncInfo(on_wait=[w], on_update=[])
                        out_list.append(nop)
                        n_split += 1
                    si.on_wait = ow[-limit:]
                    changed = True
                out_list.append(ins)
            if changed:
                b.instructions = out_list
    return n_split


def dedup_ldweights(nc):
    """Tile lowers every matmul into an explicit Ldweights + Matmult pair, so
    two consecutive matmuls sharing one stationary tile reload the PE array
    twice.  Drop an Ldweights when the instruction directly before it is a
    Matmult whose stationary operand is byte-identical and the Ldweights
    carries no semaphore waits/updates — the weights are already in the
    array."""
    n_drop = 0
    for f in nc.m.functions:
        for b in f.blocks:
            insts = list(b.instructions)
            out_list = []
            for ins in insts:
                if (type(ins).__name__ == "InstLdweights"
                        and out_list
                        and type(out_list[-1]).__name__ == "InstMatmult"
                        and len(out_list[-1].ins) >= 2
                        and str(out_list[-1].ins[1]) == str(ins.ins[0])
                        and not (ins.sync_info and ins.sync_info.on_wait)
                        and not (ins.sync_info and ins.sync_info.on_update)):
                    n_drop += 1
                    continue
                out_list.append(ins)
            if n_drop:
                b.instructions = out_list
    return n_drop


def build_nc(m_loc=M_LOC, k=K, n_loc=N_LOC, tch=TCH, n_cores=N_CORES,
             grid_m=GRID_M, split_waits=True, stage="full",
             mm_n=MM_N, dedup_ldw=True, repeat_full=1, interleave=3):
    """Build the per-core Bass graph (SPMD: identical on every core).

    repeat_full > 1 repeats the ENTIRE kernel body (gamma + AllReduce +
    quantize + matmul sweep) that many times inside one graph; test.py uses
    two repeat counts and differences the wall times to extract the per-call
    HW execution time without the host/RPC dispatch overhead.

    interleave: number of m-tiles whose PSUM accumulation chains are
    interleaved k-major during the FIRST sweep of phase B, so the PE consumes
    each q2 k-tile `interleave` times while phase A2 is still producing them
    (2 Sign ops + add per k-tile ~ 2.7 us vs 0.85 us of matmul per tile).
    """
    import concourse.bass as bass
    import concourse.mybir as mybir
    import concourse.tile as tile

    f32 = mybir.dt.float32
    bf16 = mybir.dt.bfloat16
    Alu = mybir.AluOpType
    Act = mybir.ActivationFunctionType

    tj = k // 128
    tch = min(tch, tj)
    assert tj % tch == 0
    n_ch = tj // tch
    m_tiles = m_loc // 128
    n_half = (n_loc + mm_n - 1) // mm_n
    ng = n_loc // grid_m            # gamma-slice width (disjoint across cores)
    # AR over the disjoint wg slices sums |W| exactly once; th = gamma/2
    th_scale = 1.0 / (2.0 * k * (n_loc * (n_cores // grid_m)))
    ni = max(1, min(interleave, m_tiles))

    nc = bass.Bass(num_devices=n_cores)
    # xt is host-pre-tiled: xt[mi, p, t*128+j] = x_loc[mi*128+j, t*128+p]
    # so each m-tile's load is one fully-contiguous [128, tj*128] block
    # (16 KiB runs per partition instead of 512 B strided rows).
    xt = nc.dram_tensor("xt", [m_tiles, 128, tj * 128], f32,
                        kind="ExternalInput")
    wt = nc.dram_tensor("wt", [k, n_loc], f32, kind="ExternalInput")
    wg = nc.dram_tensor("wg", [k, ng], f32, kind="ExternalInput")
    biasb = nc.dram_tensor("biasb", [128, n_loc], f32, kind="ExternalInput")
    out = nc.dram_tensor("out", [m_loc, n_loc], f32, kind="ExternalOutput")

    cc_in = nc.dram_tensor("cc_in", [1], f32, kind="Internal")
    cc_out = nc.dram_tensor("cc_out", [1], f32, kind="Internal",
                            addr_space="Shared")

    wt_r = wt[:, :].rearrange("(t p) n -> p t n", p=128)
    wg_r = wg[:, :].rearrange("(t p) n -> p t n", p=128)

    with tile.TileContext(nc) as tc:
        with (
            tc.tile_pool(name="const", bufs=1) as constp,
            tc.tile_pool(name="bias", bufs=1) as biasp,
            tc.tile_pool(name="gam", bufs=1) as gamp,
            tc.tile_pool(name="gch", bufs=3) as gchp,
            tc.tile_pool(name="wch", bufs=2) as wchp,
            tc.tile_pool(name="qtmp", bufs=2) as qtmpp,
            tc.tile_pool(name="q2", bufs=1) as q2p,
            tc.tile_pool(name="xin", bufs=2) as xinp,
            tc.tile_pool(name="xbf", bufs=ni + 1) as xbfp,
            tc.tile_pool(name="osb", bufs=2) as osbp,
            tc.tile_pool(name="ps", bufs=4, space="PSUM") as psp,
        ):
            # ---- one-time constants ----
            ones_col = constp.tile([128, 1], f32, tag="ones_col")
            nc.vector.memset(ones_col[:], 1.0)
            ones_row = constp.tile([1, 128], f32, tag="ones_row")
            nc.vector.memset(ones_row[:], 1.0)

            prev_cc = None
            prev_rd = None
            for _rep in range(repeat_full):
                # ---- phase A: gamma = max(mean|W|, 1e-8) ----
                # The SP HWDGE stream is the modeled HBM pipe (~330 GB/s);
                # everything on the gamma critical path gets chunk-wide DMAs
                # (one dma_start each, 3 bufs so slot WARs never open a gap
                # for wt/x prefetches to jump the line), and the |.|-sums
                # alternate DVE (tensor_reduce) / ACT (Abs + accum_out) so
                # reduction never paces the stream.
                n_dve = (n_ch + 1) // 2
                acc_w = n_dve + 2 * (n_ch - n_dve)
                acc = gamp.tile([128, acc_w], f32, tag="acc")
                col = 0
                for ci in range(n_ch):
                    gch = gchp.tile([128, tch * ng], f32, tag="gch")
                    gch3 = gch[:].rearrange("p (t n) -> p t n", n=ng)
                    nc.sync.dma_start(gch3[:, :, :],
                                      wg_r[:, ci * tch:(ci + 1) * tch, :])
                    if ci % 2 == 0:
                        nc.vector.tensor_reduce(
                            acc[:, col:col + 1], gch[:],
                            axis=mybir.AxisListType.X, op=Alu.add,
                            apply_absolute_value=True)
                        col += 1
                    else:
                        half = tch * ng // 2
                        for hh in range(2):
                            junk = qtmpp.tile([128, 2 * n_loc], bf16,
                                              tag="qa")
                            nc.scalar.activation(
                                junk[:, 0:half],
                                gch[:, hh * half:(hh + 1) * half],
                                Act.Abs, accum_out=acc[:, col:col + 1])
                            col += 1
                acc1 = gamp.tile([128, 1], f32, tag="acc1")
                nc.vector.tensor_reduce(acc1[:], acc[:],
                                        axis=mybir.AxisListType.X, op=Alu.add)
                # cross-partition sum -> [1, 1] (PSUM, bank borrowed from ps)
                pA = psp.tile([128, n_loc], f32, tag="ps")
                nc.tensor.matmul(pA[0:1, 0:1], lhsT=acc1[:], rhs=ones_col[:],
                                 start=True, stop=True)
                s_sb = gamp.tile([1, 1], f32, tag="s_sb")
                nc.vector.tensor_copy(s_sb[:], pA[0:1, 0:1])
                # the cc handoff DMAs ride the ACT HWDGE stream: SP may be
                # mid-way through a multi-us wt/x chunk right now
                wr = nc.scalar.dma_start(cc_in[0:1], s_sb[0:1, 0])
                if prev_cc is not None:
                    tile.add_dep_helper(wr.ins, prev_cc.ins,
                                        reason="WAR cc_in across reps")
                cc = nc.gpsimd.collective_compute(
                    "AllReduce", Alu.add,
                    replica_groups=[list(range(n_cores))],
                    ins=[cc_in.ap().opt()], outs=[cc_out.ap().opt()])
                if prev_rd is not None:
                    tile.add_dep_helper(cc.ins, prev_rd.ins,
                                        reason="WAR cc_out across reps")
                s2_sb = gamp.tile([1, 1], f32, tag="s2_sb")
                rd = nc.scalar.dma_start(s2_sb[0:1, 0], cc_out[0:1])
                tile.add_dep_helper(rd.ins, cc.ins, reason="read AR result")
                prev_cc, prev_rd = cc, rd
                # broadcast to all 128 partitions
                pB = psp.tile([128, n_loc], f32, tag="ps")
                nc.tensor.matmul(pB[:, 0:1], lhsT=ones_row[:], rhs=s2_sb[:],
                                 start=True, stop=True)
                th = gamp.tile([128, 1], f32, tag="th")
                nth = gamp.tile([128, 1], f32, tag="nth")
                nc.vector.tensor_scalar(th[:], pB[:, 0:1], th_scale, TH_FLOOR,
                                        op0=Alu.mult, op1=Alu.max)
                nc.vector.tensor_scalar(nth[:], pB[:, 0:1], -th_scale,
                                        -TH_FLOOR, op0=Alu.mult, op1=Alu.min)

                biasb_sb = biasp.tile([128, n_loc], f32, tag="biasb")
                nc.scalar.dma_start(biasb_sb[:], biasb[:, :])

                # ---- phase A2: quantize W -> q2 = 2*qw (bf16, resident) ----
                # k-tile-pair chunks: Sign activations batched over 2 tiles
                # (fewer ACT instructions) and small enough chunks that the
                # tiny cc_in write never waits long behind a wt chunk on SP.
                q2 = q2p.tile([128, tj * n_loc], bf16, tag="q2")
                q2_3 = q2[:].rearrange("p (t n) -> p t n", n=n_loc)
                pair = 2 * n_loc
                for ci in range(tj // 2):
                    wch = wchp.tile([128, pair], f32, tag="wg")
                    nc.sync.dma_start(
                        wch[:].rearrange("p (t n) -> p t n", n=n_loc),
                        wt_r[:, ci * 2:(ci + 1) * 2, :])
                    a = qtmpp.tile([128, pair], bf16, tag="qa")
                    b = qtmpp.tile([128, pair], bf16, tag="qb")
                    nc.scalar.activation(a[:], wch[:], Act.Sign,
                                         bias=nth[:], scale=1.0)
                    nc.scalar.activation(b[:], wch[:], Act.Sign,
                                         bias=th[:], scale=1.0)
                    nc.vector.tensor_tensor(
                        q2[:, ci * pair:(ci + 1) * pair], a[:], b[:],
                        op=Alu.add)

                if stage == "prologue":
                    osb = osbp.tile([128, n_loc], f32, tag="osb")
                    nc.vector.tensor_tensor(osb[:], biasb_sb[:],
                                            biasb_sb[:], op=Alu.add)
                    nc.sync.dma_start(out[0:128, :], osb[:])
                    continue

                # ---- phase B: out = (0.5 x)T q2 + bias ----
                def load_cast(mi):
                    xraw = xinp.tile([128, tj * 128], f32, tag="xraw")
                    # 2 dma_starts per 2 MB tile: balances SP sequencer time
                    # (565 ns each) against per-queue bandwidth
                    half = (tj * 128) // 2
                    for c in range(2):
                        nc.sync.dma_start(
                            xraw[:, c * half:(c + 1) * half],
                            xt[mi, :, c * half:(c + 1) * half])
                    xbf = xbfp.tile([128, tj * 128], bf16, tag="xbf")
                    nc.scalar.activation(xbf[:], xraw[:], Act.Copy, scale=0.5)
                    return xbf[:].rearrange("p (t j) -> p t j", j=128)

                def store(mi, ps):
                    osb = osbp.tile([128, n_loc], f32, tag="osb")
                    nc.vector.tensor_tensor(osb[:], ps[:], biasb_sb[:],
                                            op=Alu.add)
                    # output stores ride the ACT HWDGE stream, keeping the
                    # SP stream free for the x/w input loads
                    nc.scalar.dma_start(out[mi * 128:(mi + 1) * 128, :],
                                        osb[:])

                # first sweep: `ni` m-tiles interleaved k-major so the PE
                # keeps up with A2's q2 production rate
                xbfs = [load_cast(mi) for mi in range(ni)]
                pss = []
                for mi in range(ni):
                    psi = psp.tile([128, n_loc], f32, tag="ps")
                    pss.append(psi)
                for t in range(tj):
                    for mi in range(ni):
                        for h in range(n_half):
                            n0 = h * mm_n
                            n1 = min(n_loc, n0 + mm_n)
                            nc.tensor.matmul(pss[mi][:, n0:n1],
                                             lhsT=xbfs[mi][:, t, :],
                                             rhs=q2_3[:, t, n0:n1],
                                             start=(t == 0),
                                             stop=(t == tj - 1))
                for mi in range(ni):
                    store(mi, pss[mi])

                # remaining m-tiles: k-inner, one PSUM chain each
                for mi in range(ni, m_tiles):
                    xbf3 = load_cast(mi)
                    ps = psp.tile([128, n_loc], f32, tag="ps")
                    for t in range(tj):
                        for h in range(n_half):
                            n0 = h * mm_n
                            n1 = min(n_loc, n0 + mm_n)
                            nc.tensor.matmul(ps[:, n0:n1],
                                             lhsT=xbf3[:, t, :],
                                             rhs=q2_3[:, t, n0:n1],
                                             start=(t == 0),
                                             stop=(t == tj - 1))
                    store(mi, ps)

    if dedup_ldw:
        dedup_ldweights(nc)
    if split_waits:
        split_multi_waits(nc)
    return nc


def shard_inputs(x, weight, bias, m_loc=M_LOC, n_loc=N_LOC, n_cores=N_CORES,
                 grid_n=GRID_N):
    """Host-side layout prep (transpose/slice/broadcast only)."""
    x2 = np.ascontiguousarray(x.reshape(-1, x.shape[-1]))     # [M, K]
    k = x2.shape[1]
    m_tiles, tj = m_loc // 128, k // 128
    grid_m = n_cores // grid_n
    ng = n_loc // grid_m
    in_maps = []
    xts = {}
    for c in range(n_cores):
        mi, ni = c // grid_n, c % grid_n
        if mi not in xts:
            # xt[mi, p, t*128+j] = x_loc[mi*128+j, t*128+p]
            xl = x2[mi * m_loc:(mi + 1) * m_loc, :]
            xts[mi] = np.ascontiguousarray(
                xl.reshape(m_tiles, 128, tj, 128)
                .transpose(0, 3, 2, 1)
                .reshape(m_tiles, 128, tj * 128))
        wt = np.ascontiguousarray(weight[ni * n_loc:(ni + 1) * n_loc, :].T)
        g0 = ni * n_loc + mi * ng
        wgt = np.ascontiguousarray(weight[g0:g0 + ng, :].T)
        bb = np.ascontiguousarray(
            np.broadcast_to(bias[ni * n_loc:(ni + 1) * n_loc], (128, n_loc)))
        in_maps.append({"xt": xts[mi], "wt": wt, "wg": wgt, "biasb": bb})
    return in_maps


def unshard_output(outs, x_shape, m_loc=M_LOC, n_loc=N_LOC, n_cores=N_CORES,
                   grid_m=GRID_M, grid_n=GRID_N):
    n = grid_n * n_loc
    full = np.empty((grid_m * m_loc, n), dtype=outs[0].dtype)
    for c in range(n_cores):
        mi, ni = c // grid_n, c % grid_n
        full[mi * m_loc:(mi + 1) * m_loc, ni * n_loc:(ni + 1) * n_loc] = outs[c]
    return full.reshape(*x_shape[:-1], n)


def kernel(x, weight, bias):
    from concourse.bass_utils import run_bass_kernel_spmd

    nc = build_nc()
    in_maps = shard_inputs(x, weight, bias)
    res = run_bass_kernel_spmd(nc, in_maps, core_ids=list(range(N_CORES)))
    outs = [res.results[c]["out"] for c in range(N_CORES)]
    return unshard_output(outs, x.shape)

